# revision 1
# baseline (speedup 1.0000x reference)
"""Trainium2 Bass kernel for nn_Attention_84585085927925 — Gram variant.

Reference (per batch element b, all fp32):
    qkv = x @ w_qkv.T ; q,k,v heads of 64 ; attn = sqrt(64) * q @ k.T (NO
    softmax) ; out = attn @ v ; out = out @ w_fc.T + b_fc

With no softmax the attention is associative, and k/v can be folded into
the weights via the Gram matrix of x:
    out_h = (s*q_h) @ (k_h.T @ v_h) = (s*q_h) @ (wk_h @ (x.T x) @ wv_h.T)
Per-core pipeline (one batch element per NeuronCore, 8 cores, no
collectives; e = output-feature axis, d = input-feature axis):
    qT   = (s*w_q).T-stationary @ xT       -> [768,1024]
    C    = x.T x  (x-stationary)           -> [768,768] (symmetric)
    T1   = C-stationary @ wv.T             -> [768,768]
    G    = wk.T-stationary @ T1 per pair   -> block-diag [128,128] per pair
    aoT  = G2-stationary @ qT per pair     -> [768,1024]
    outT = w_fc.T-stationary @ aoT + b_fc  -> [768,1024]
Host transposes x and outT. Matmuls run in float32r (~4x faster than
fp32, ~3e-4 relative error).
"""

import numpy as np

import concourse.bass as bass  # noqa: F401  (registers engine namespaces)
import concourse.mybir as mybir
import concourse.tile as tile
from concourse import bacc, bass_utils

F32 = mybir.dt.float32
F32R = mybir.dt.float32r

B, N, D, H = 8, 1024, 768, 12
HD = D // H            # 64
SCALE = float(np.sqrt(HD))
DT = D // 128          # 6  d-tiles
ET = D // 128          # 6  e-tiles
NT = N // 128          # 8  n(token)-tiles
NC2 = N // 512         # 2  512-wide token chunks
ECH = 384              # e-chunk that fits one PSUM bank with headroom
NPAIR = H // 2         # 6 head pairs


def _build_program():
    nc = bacc.Bacc(
        trn_type="TRN2", target_bir_lowering=False, debug=False, num_devices=B
    )
    xT_d = nc.dram_tensor("xT", [D, N], F32, kind="ExternalInput").ap()
    xN_d = nc.dram_tensor("xN", [N, D], F32, kind="ExternalInput").ap()
    wqT_d = nc.dram_tensor("wqT", [D, D], F32, kind="ExternalInput").ap()
    wkT_d = nc.dram_tensor("wkT", [D, D], F32, kind="ExternalInput").ap()
    wvT_d = nc.dram_tensor("wvT", [D, D], F32, kind="ExternalInput").ap()
    wfcT_d = nc.dram_tensor("wfcT", [D, D], F32, kind="ExternalInput").ap()
    bfc_d = nc.dram_tensor("bfc", [D], F32, kind="ExternalInput").ap()
    outT_d = nc.dram_tensor("outT", [D, N], F32, kind="ExternalOutput").ap()

    with tile.TileContext(nc) as tc:
        with tc.tile_pool(name="big", bufs=1) as big, \
             tc.tile_pool(name="wsp", bufs=3) as wsp, \
             tc.tile_pool(name="outsp", bufs=6) as outsp, \
             tc.tile_pool(name="ps", bufs=6, space="PSUM") as ps, \
             tc.tile_pool(name="psg", bufs=2, space="PSUM") as psg:

            xT_sb = big.tile([128, DT, N], F32R, name="xT_sb")
            xN_sb = big.tile([128, NT, D], F32R, name="xN_sb")
            qT_sb = big.tile([128, ET, N], F32R, name="qT_sb")
            c_sb = big.tile([128, DT, D], F32R, name="c_sb")
            t1_sb = big.tile([128, DT, D], F32R, name="t1_sb")
            ao_sb = big.tile([128, DT, N], F32R, name="ao_sb")
            g2_sb = big.tile([128, NPAIR, 128], F32R, name="g2_sb")
            bias_sb = big.tile([128, ET], F32, name="bias_sb")

            wq_r = wqT_d.rearrange("(o p) e -> p o e", p=128).bitcast(F32R)
            xT_r = xT_d.rearrange("(o p) n -> p o n", p=128).bitcast(F32R)
            xN_r = xN_d.rearrange("(o p) e -> p o e", p=128).bitcast(F32R)

            wq_tiles = []
            for et in range(ET):
                wq_t = wsp.tile([128, DT, 128], F32R, tag="w128", bufs=7,
                                name=f"wq_t{et}", uniquify=False)
                wq_tiles.append(wq_t)
            # first-needed data first: wq0 halves, xT n-half 0, then the rest
            for dh in range(2):
                dsl = slice(dh * 3, dh * 3 + 3)
                nc.sync.dma_start(wq_tiles[0][:, dsl, :], wq_r[:, dsl, 0:128])
            for dt in range(DT):
                nc.sync.dma_start(xT_sb[:, dt, 0:512], xT_r[:, dt, 0:512])
            for et in range(1, ET):
                for dh in range(2):
                    dsl = slice(dh * 3, dh * 3 + 3)
                    nc.sync.dma_start(wq_tiles[et][:, dsl, :],
                                      wq_r[:, dsl, et * 128:(et + 1) * 128])
            for dt in range(DT):
                nc.sync.dma_start(xT_sb[:, dt, 512:1024], xT_r[:, dt, 512:1024])
            for nt in range(NT):
                nc.sync.dma_start(xN_sb[:, nt, :], xN_r[:, nt, :])
            nc.sync.dma_start(bias_sb[:],
                              bfc_d.rearrange("(o p) -> p o", p=128))

            # ---- q.T projection: lhsT = wqT tile [d,e], rhs = xT chunk ----
            qt_chunks = [(0, 0, 256), (0, 256, 256),
                         (1, 0, 512), (2, 0, 512), (3, 0, 512),
                         (4, 0, 512), (5, 0, 512),
                         (0, 512, 512), (1, 512, 512), (2, 512, 512),
                         (3, 512, 512), (4, 512, 512), (5, 512, 512)]
            for et, off, width in qt_chunks:
                wq_t = wq_tiles[et]
                pt = ps.tile([128, 512], F32, tag="ps", name="pt_q")
                for dt in range(DT):
                    nc.tensor.matmul(
                        pt[:, :width],
                        wq_t[:, dt, :],
                        xT_sb[:, dt, off:off + width],
                        start=(dt == 0), stop=(dt == DT - 1),
                    )
                nc.vector.tensor_copy(
                    qT_sb[:, et, off:off + width], pt[:, :width]
                )

            # ---- C = x.T x : lhsT = x tile [n, d1], rhs = x [n, d2-chunk] --
            for ec in range(D // ECH):
                for d1t in range(DT):
                    pt = ps.tile([128, ECH], F32, tag="ps", name="pt_c")
                    for nt in range(NT):
                        nc.tensor.matmul(
                            pt[:],
                            xN_sb[:, nt, d1t * 128:(d1t + 1) * 128],
                            xN_sb[:, nt, ec * ECH:(ec + 1) * ECH],
                            start=(nt == 0), stop=(nt == NT - 1),
                        )
                    nc.vector.tensor_copy(
                        c_sb[:, d1t, ec * ECH:(ec + 1) * ECH], pt[:]
                    )

            # ---- T1 = C @ wv.T : lhsT = C tile (symmetric), rhs = wvT ----
            wv_r = wvT_d.rearrange("(o p) e -> p o e", p=128).bitcast(F32R)
            for ec in range(D // ECH):
                wv_t = wsp.tile([128, DT, ECH], F32R, tag="w384",
                                name=f"wv_t{ec}", uniquify=False)
                for dh in range(3):
                    dsl = slice(dh * 2, dh * 2 + 2)
                    nc.sync.dma_start(
                        wv_t[:, dsl, :],
                        wv_r[:, dsl, ec * ECH:(ec + 1) * ECH],
                    )
                for d1t in range(DT):
                    pt = ps.tile([128, ECH], F32, tag="ps", name="pt_t1")
                    for d2t in range(DT):
                        nc.tensor.matmul(
                            pt[:],
                            c_sb[:, d2t, d1t * 128:(d1t + 1) * 128],
                            wv_t[:, d2t, :],
                            start=(d2t == 0), stop=(d2t == DT - 1),
                        )
                    nc.vector.tensor_copy(
                        t1_sb[:, d1t, ec * ECH:(ec + 1) * ECH], pt[:]
                    )

            # ---- G = wk @ T1 per head pair, stored block-diagonal ----
            wk_r = wkT_d.rearrange("(o p) e -> p o e", p=128).bitcast(F32R)
            for t in range(NPAIR):
                wk_t = wsp.tile([128, DT, 128], F32R, tag="w128", bufs=7,
                                name=f"wk_t{t}", uniquify=False)
                nc.sync.dma_start(wk_t[:], wk_r[:, :, t * 128:(t + 1) * 128])
                gp = psg.tile([128, 128], F32, tag="psg", name="gp")
                for dt in range(DT):
                    nc.tensor.matmul(
                        gp[:],
                        wk_t[:, dt, :],
                        t1_sb[:, dt, t * 128:(t + 1) * 128],
                        start=(dt == 0), stop=(dt == DT - 1),
                    )
                nc.vector.tensor_scalar_mul(g2_sb[:, t, :], gp[:], 0.0)
                nc.vector.tensor_copy(g2_sb[0:64, t, 0:64], gp[0:64, 0:64])
                nc.vector.tensor_copy(g2_sb[64:128, t, 64:128],
                                      gp[64:128, 64:128])

            # ---- attn-out.T then fc, interleaved per 512-chunk ----
            wfc_r = wfcT_d.rearrange("(o p) e -> p o e", p=128).bitcast(F32R)
            wfc_tiles = []
            for et in range(ET):
                wfc_t = wsp.tile([128, DT, 128], F32R, tag="w128", bufs=7,
                                 name=f"wfc_t{et}", uniquify=False)
                wfc_tiles.append(wfc_t)
                nc.sync.dma_start(wfc_t[:], wfc_r[:, :, et * 128:(et + 1) * 128])
            for ic in range(NC2):
                for t in range(NPAIR):
                    pt = ps.tile([128, 512], F32, tag="ps", name="pt_ao")
                    nc.tensor.matmul(
                        pt[:],
                        g2_sb[:, t, :],
                        qT_sb[:, t, ic * 512:(ic + 1) * 512],
                        start=True, stop=True,
                    )
                    dst_ap = ao_sb[:, t, ic * 512:(ic + 1) * 512]
                    if t % 2 == 0:
                        nc.vector.tensor_copy(dst_ap, pt[:])
                    else:
                        nc.scalar.copy(dst_ap, pt[:])
                for et in range(ET):
                    wfc_t = wfc_tiles[et]
                    pt = ps.tile([128, 512], F32, tag="ps", name="pt_fc")
                    for dt in range(DT):
                        nc.tensor.matmul(
                            pt[:],
                            wfc_t[:, dt, :],
                            ao_sb[:, dt, ic * 512:(ic + 1) * 512],
                            start=(dt == 0), stop=(dt == DT - 1),
                        )
                    ot = outsp.tile([128, 512], F32, tag="ot", name="ot")
                    nc.scalar.add(ot[:], pt[:], bias_sb[:, et:et + 1])
                    last = (ic == NC2 - 1 and et == ET - 1)
                    nsplit = 4 if last else 1
                    w = 128 // nsplit
                    for ph in range(nsplit):
                        nc.sync.dma_start(
                            outT_d[et * 128 + ph * w:et * 128 + (ph + 1) * w,
                                   ic * 512:(ic + 1) * 512],
                            ot[ph * w:(ph + 1) * w, :],
                        )

    nc.compile()
    return nc


_NC_CACHE = None
LAST_EXEC_NS = None
LAST_RES = None


def kernel(x, w_qkv, w_fc, b_fc, _trace=False):
    global _NC_CACHE, LAST_EXEC_NS, LAST_RES
    x = np.asarray(x, dtype=np.float32)
    w_qkv = np.asarray(w_qkv, dtype=np.float32)
    w_fc = np.asarray(w_fc, dtype=np.float32)
    b_fc = np.asarray(b_fc, dtype=np.float32)

    if _NC_CACHE is None:
        _NC_CACHE = _build_program()
    nc = _NC_CACHE

    wqT = np.ascontiguousarray((SCALE * w_qkv[:D]).T)
    wkT = np.ascontiguousarray(w_qkv[D:2 * D].T)
    wvT = np.ascontiguousarray(w_qkv[2 * D:].T)
    wfcT = np.ascontiguousarray(w_fc.T)

    in_maps = []
    for b in range(B):
        in_maps.append({
            "xT": np.ascontiguousarray(x[b].T),
            "xN": np.ascontiguousarray(x[b]),
            "wqT": wqT, "wkT": wkT, "wvT": wvT, "wfcT": wfcT,
            "bfc": b_fc,
        })

    res = bass_utils.run_bass_kernel_spmd(
        nc, in_maps, core_ids=list(range(B)), trace=_trace
    )
    LAST_EXEC_NS = res.exec_time_ns
    LAST_RES = res
    out = np.stack([res.results[b]["outT"].T for b in range(B)])
    return np.ascontiguousarray(out.astype(np.float32))



# revision 4
# speedup vs baseline: 1.3472x; 1.3472x over previous
"""Trainium2 Bass kernel for nn_Attention_84585085927925 — W_eff variant.

Reference (per batch element b, all fp32):
    qkv = x @ w_qkv.T ; q,k,v heads of 64 ; attn = sqrt(64) * q @ k.T (NO
    softmax) ; out = attn @ v ; out = out @ w_fc.T + b_fc

With no softmax the whole module is linear in x on the left:
    out = x @ W_eff + b_fc,
    W_eff = sum_h (s*w_q_h)^T (w_k_h C w_v_h^T) (w_fc^T)_h,   C = x^T x
so q/k/v are never materialized. Per-core pipeline (one batch element per
NeuronCore, 8 cores, no collectives), everything bf16 into fp32 PSUM:
    C    = x^T x           upper blocks only; lower via PE transpose
    T1   = C @ wv^T        [768,768]
    GT_t = T1_t^T @ wkT_t  per head pair; keep diagonal 64x64 blocks
    M_t  = G2_t @ F_t      F = wfc^T rows; block-diag stationary
    Weff = (s*wq)^T @ M    [768,768]
    outT = Weff^T x^T + b  [768,1024] -> bf16 out
Host does layout transposes + bf16 casts. ~125K PE cycles/core.

Scheduling: xN arrives in 4 chunks ([1,1,2,4] n-tiles); C runs nt-outer
in 3 row-pass groups so the PE consumes chunks in arrival order; PE
warms its p-state on identity transposes while the first chunk is in
flight; GT/M are interleaved; the last out row-block is split
512/384/128 so the final dependent DMA is small.
"""

import numpy as np
import ml_dtypes

import concourse.bass as bass  # noqa: F401  (registers engine namespaces)
import concourse.mybir as mybir
import concourse.tile as tile
from concourse import bacc, bass_utils

F32 = mybir.dt.float32
BF16 = mybir.dt.bfloat16
NPBF16 = ml_dtypes.bfloat16

B, N, D, H = 8, 1024, 768, 12
HD = D // H            # 64
SCALE = float(np.sqrt(HD))
DT = D // 128          # 6  d-tiles
NT = N // 128          # 8  n(token)-tiles
NPAIR = H // 2         # 6 head pairs

XN_CHUNKS = [(0, 1), (1, 1), (2, 2), (4, 4)]   # (start nt, n nt)
C_PASSES = [(0, 1), (2, 3), (4, 5)]            # row groups per nt sweep
N_WARMUP = 24


def _chunks(lo, hi, step=512):
    out = []
    while lo < hi:
        out.append((lo, min(step, hi - lo)))
        lo += step
    return out


def _build_program():
    nc = bacc.Bacc(
        trn_type="TRN2", target_bir_lowering=False, debug=False, num_devices=B
    )
    xN_d = nc.dram_tensor("xN", [N, D], BF16, kind="ExternalInput").ap()
    xT_d = nc.dram_tensor("xT", [D, N], BF16, kind="ExternalInput").ap()
    wq_d = nc.dram_tensor("wq", [D, D], BF16, kind="ExternalInput").ap()
    wvT_d = nc.dram_tensor("wvT", [D, D], BF16, kind="ExternalInput").ap()
    wkT_d = nc.dram_tensor("wkT", [D, D], BF16, kind="ExternalInput").ap()
    wfcT_d = nc.dram_tensor("wfcT", [D, D], BF16, kind="ExternalInput").ap()
    bfc_d = nc.dram_tensor("bfc", [D], F32, kind="ExternalInput").ap()
    ident_d = nc.dram_tensor("ident", [128, 128], BF16, kind="ExternalInput").ap()
    outT_d = nc.dram_tensor("outT", [D, N], BF16, kind="ExternalOutput").ap()

    xN_r = xN_d.rearrange("(o p) d -> p o d", p=128)
    xT_r = xT_d.rearrange("(o p) n -> p o n", p=128)
    wq_r = wq_d.rearrange("(o p) d -> p o d", p=128)
    wvT_r = wvT_d.rearrange("(o p) j -> p o j", p=128)
    wkT_r = wkT_d.rearrange("(o p) j -> p o j", p=128)
    wfcT_r = wfcT_d.rearrange("(o p) e -> p o e", p=128)
    outT_r = outT_d.rearrange("(o p) n -> p o n", p=128)

    with tile.TileContext(nc) as tc:
        with tc.tile_pool(name="big", bufs=1) as big, \
             tc.tile_pool(name="outsp", bufs=3) as outsp, \
             tc.tile_pool(name="ps", bufs=6, space="PSUM") as ps, \
             tc.tile_pool(name="tp", bufs=2, space="PSUM") as tp:

            xN_sb = big.tile([128, NT, D], BF16, name="xN_sb")
            xT_sb = big.tile([128, DT, N], BF16, name="xT_sb")
            c_sb = big.tile([128, DT, D], BF16, name="c_sb")
            t1_sb = big.tile([128, DT, D], BF16, name="t1_sb")
            g2T_sb = big.tile([128, NPAIR, 128], BF16, name="g2T_sb")
            m_sb = big.tile([128, NPAIR, D], BF16, name="m_sb")
            weff_sb = big.tile([128, DT, D], BF16, name="weff_sb")
            wq_sb = big.tile([128, DT, D], BF16, name="wq_sb")
            wvT_sb = big.tile([128, DT, D], BF16, name="wvT_sb")
            wkT_sb = big.tile([128, DT, D], BF16, name="wkT_sb")
            wfcT_sb = big.tile([128, NPAIR, D], BF16, name="wfcT_sb")
            bias_sb = big.tile([128, DT], F32, name="bias_sb")
            ident_sb = big.tile([128, 128], BF16, name="ident_sb")

            # ---- input DMAs: tiny ident first (PE warmup), then xN ----
            nc.sync.dma_start(ident_sb[:], ident_d)
            for nt0, nn in XN_CHUNKS:
                nc.sync.dma_start(xN_sb[:, nt0:nt0 + nn, :],
                                  xN_r[:, nt0:nt0 + nn, :])
            nc.sync.dma_start(wvT_sb[:], wvT_r)
            nc.sync.dma_start(wkT_sb[:], wkT_r)
            nc.sync.dma_start(wfcT_sb[:], wfcT_r)
            nc.sync.dma_start(wq_sb[:], wq_r)
            nc.sync.dma_start(bias_sb[:], bfc_d.rearrange("(o p) -> p o", p=128))
            nc.sync.dma_start(xT_sb[:], xT_r)

            nc.vector.memset(g2T_sb[:], 0.0)

            # ---- PE p-state warmup while waiting for the first xN chunk ---
            for _ in range(N_WARMUP):
                wt = tp.tile([128, 128], BF16, tag="tp", name="wt")
                nc.tensor.transpose(wt[:], ident_sb[:], ident_sb[:])

            # ---- C = x^T x, upper blocks; nt-outer passes over row groups -
            # c_sb[p, i, c] = C[i*128+p, c]
            row_cols = {i: _chunks(i * 128, D) for i in range(DT)}
            row_psum = {}
            for rows in C_PASSES:
                for i in rows:
                    row_psum[i] = [
                        ps.tile([128, w], F32, tag="bank", name=f"pt_c{i}")
                        for (_, w) in row_cols[i]
                    ]
                for nt in range(NT):
                    for i in rows:
                        lhs = xN_sb[:, nt, i * 128:(i + 1) * 128]
                        for (off, w), pt in zip(row_cols[i], row_psum[i]):
                            nc.tensor.matmul(
                                pt[:], lhs, xN_sb[:, nt, off:off + w],
                                start=(nt == 0), stop=(nt == NT - 1),
                            )
                for i in rows:
                    for (off, w), pt in zip(row_cols[i], row_psum[i]):
                        nc.vector.tensor_copy(c_sb[:, i, off:off + w], pt[:])

            # lower block (j, i) = transpose of upper (i, j), i < j
            for i in range(DT):
                for j in range(i + 1, DT):
                    tpt = tp.tile([128, 128], BF16, tag="tp", name="tp_c")
                    nc.tensor.transpose(
                        tpt[:], c_sb[:, i, j * 128:(j + 1) * 128], ident_sb[:]
                    )
                    nc.vector.tensor_copy(
                        c_sb[:, j, i * 128:(i + 1) * 128], tpt[:]
                    )

            # ---- T1 = C @ wv^T : T1[d1, j'] = sum_d2 C[d2, d1] wvT[d2, j'] --
            for d1 in range(DT):
                cols = _chunks(0, D)
                pts = [ps.tile([128, w], F32, tag="bank", name="pt_t1")
                       for (_, w) in cols]
                for d2 in range(DT):
                    lhs = c_sb[:, d2, d1 * 128:(d1 + 1) * 128]
                    for (off, w), pt in zip(cols, pts):
                        nc.tensor.matmul(
                            pt[:], lhs, wvT_sb[:, d2, off:off + w],
                            start=(d2 == 0), stop=(d2 == DT - 1),
                        )
                for k, ((off, w), pt) in enumerate(zip(cols, pts)):
                    eng = nc.scalar if k == 0 else nc.vector
                    (eng.copy if k == 0 else eng.tensor_copy)(
                        t1_sb[:, d1, off:off + w], pt[:])

            # ---- GT per pair (gt = T1_pair^T @ wkT_pair), M one pair behind
            def emit_gt(t):
                gt = tp.tile([128, 128], F32, tag="tp", name="gt")
                pc = slice(t * 128, (t + 1) * 128)
                for dt in range(DT):
                    nc.tensor.matmul(
                        gt[:], t1_sb[:, dt, pc], wkT_sb[:, dt, pc],
                        start=(dt == 0), stop=(dt == DT - 1),
                    )
                nc.vector.tensor_copy(g2T_sb[0:64, t, 0:64], gt[0:64, 0:64])
                nc.vector.tensor_copy(g2T_sb[64:128, t, 64:128],
                                      gt[64:128, 64:128])

            def emit_m(t):
                cols = _chunks(0, D)
                for k, (off, w) in enumerate(cols):
                    pm = ps.tile([128, w], F32, tag="bank", name="pt_m")
                    nc.tensor.matmul(
                        pm[:], g2T_sb[:, t, :], wfcT_sb[:, t, off:off + w],
                        start=True, stop=True,
                    )
                    if k == 0:
                        nc.scalar.copy(m_sb[:, t, off:off + w], pm[:])
                    else:
                        nc.vector.tensor_copy(m_sb[:, t, off:off + w], pm[:])

            emit_gt(0)
            for t in range(1, NPAIR):
                emit_gt(t)
                emit_m(t - 1)
            emit_m(NPAIR - 1)

            # ---- Weff[d, e] = sum_j wq_s[j, d] M[j, e] ----
            for dt in range(DT):
                cols = _chunks(0, D)
                pts = [ps.tile([128, w], F32, tag="bank", name="pt_w")
                       for (_, w) in cols]
                for jt in range(DT):
                    lhs = wq_sb[:, jt, dt * 128:(dt + 1) * 128]
                    for (off, w), pt in zip(cols, pts):
                        nc.tensor.matmul(
                            pt[:], lhs, m_sb[:, jt, off:off + w],
                            start=(jt == 0), stop=(jt == DT - 1),
                        )
                for k, ((off, w), pt) in enumerate(zip(cols, pts)):
                    if k == 0:
                        nc.scalar.copy(weff_sb[:, dt, off:off + w], pt[:])
                    else:
                        nc.vector.tensor_copy(weff_sb[:, dt, off:off + w],
                                              pt[:])

            # ---- outT[e, n] = sum_d Weff[d, e] xT[d, n] + b[e] ----
            for et in range(DT):
                last = (et == DT - 1)
                cols = [(0, 512), (512, 384), (896, 128)] if last \
                    else [(0, 512), (512, 512)]
                pts = [ps.tile([128, w], F32, tag="bank", name="pt_o")
                       for (_, w) in cols]
                for dt in range(DT):
                    lhs = weff_sb[:, dt, et * 128:(et + 1) * 128]
                    for (off, w), pt in zip(cols, pts):
                        nc.tensor.matmul(
                            pt[:], lhs, xT_sb[:, dt, off:off + w],
                            start=(dt == 0), stop=(dt == DT - 1),
                        )
                ot = outsp.tile([128, N], BF16, tag="ot", name="ot")
                for k, ((off, w), pt) in enumerate(zip(cols, pts)):
                    if last and k == 1:
                        nc.vector.tensor_scalar_add(
                            ot[:, off:off + w], pt[:], bias_sb[:, et:et + 1])
                    else:
                        nc.scalar.add(ot[:, off:off + w], pt[:],
                                      bias_sb[:, et:et + 1])
                    if last:
                        nc.sync.dma_start(outT_r[:, et, off:off + w],
                                          ot[:, off:off + w])
                if not last:
                    nc.sync.dma_start(outT_r[:, et, :], ot[:])

    nc.compile()
    return nc


_NC_CACHE = None
LAST_EXEC_NS = None
LAST_RES = None


def kernel(x, w_qkv, w_fc, b_fc, _trace=False):
    global _NC_CACHE, LAST_EXEC_NS, LAST_RES
    x = np.asarray(x, dtype=np.float32)
    w_qkv = np.asarray(w_qkv, dtype=np.float32)
    w_fc = np.asarray(w_fc, dtype=np.float32)
    b_fc = np.asarray(b_fc, dtype=np.float32)

    if _NC_CACHE is None:
        _NC_CACHE = _build_program()
    nc = _NC_CACHE

    wq = np.ascontiguousarray(SCALE * w_qkv[:D]).astype(NPBF16)
    wkT = np.ascontiguousarray(w_qkv[D:2 * D].T).astype(NPBF16)
    wvT = np.ascontiguousarray(w_qkv[2 * D:].T).astype(NPBF16)
    wfcT = np.ascontiguousarray(w_fc.T).astype(NPBF16)
    ident = np.eye(128, dtype=NPBF16)

    in_maps = []
    for b in range(B):
        in_maps.append({
            "xN": x[b].astype(NPBF16),
            "xT": np.ascontiguousarray(x[b].T).astype(NPBF16),
            "wq": wq, "wkT": wkT, "wvT": wvT, "wfcT": wfcT,
            "bfc": b_fc, "ident": ident,
        })

    res = bass_utils.run_bass_kernel_spmd(
        nc, in_maps, core_ids=list(range(B)), trace=_trace
    )
    LAST_EXEC_NS = res.exec_time_ns
    LAST_RES = res
    out = np.stack([res.results[b]["outT"].T.astype(np.float32)
                    for b in range(B)])
    return np.ascontiguousarray(out)


# revision 5
# speedup vs baseline: 1.3524x; 1.0038x over previous
"""Trainium2 Bass kernel for nn_Attention_84585085927925 — W_eff variant.

Reference (per batch element b, all fp32):
    qkv = x @ w_qkv.T ; q,k,v heads of 64 ; attn = sqrt(64) * q @ k.T (NO
    softmax) ; out = attn @ v ; out = out @ w_fc.T + b_fc

With no softmax the whole module is linear in x on the left:
    out = x @ W_eff + b_fc,
    W_eff = sum_h (s*w_q_h)^T (w_k_h C w_v_h^T) (w_fc^T)_h,   C = x^T x
so q/k/v are never materialized. Per-core pipeline (one batch element per
NeuronCore, 8 cores, no collectives), everything bf16 into fp32 PSUM:
    C    = x^T x           upper blocks only; lower via PE transpose
    T1   = C @ wv^T        [768,768]
    GT_t = T1_t^T @ wkT_t  per head pair; keep diagonal 64x64 blocks
    M_t  = G2_t @ F_t      F = wfc^T rows; block-diag stationary
    Weff = (s*wq)^T @ M    [768,768]
    outT = Weff^T x^T + b  [768,1024] -> bf16 out
Host does layout transposes + bf16 casts. ~125K PE cycles/core.

Scheduling: xN arrives in 4 chunks ([1,1,2,4] n-tiles); C runs nt-outer
in 3 row-pass groups so the PE consumes chunks in arrival order; PE
warms its p-state on identity transposes while the first chunk is in
flight; GT/M are interleaved; the last out row-block is split
512/384/128 so the final dependent DMA is small.
"""

import numpy as np
import ml_dtypes

import concourse.bass as bass  # noqa: F401  (registers engine namespaces)
import concourse.mybir as mybir
import concourse.tile as tile
from concourse import bacc, bass_utils

F32 = mybir.dt.float32
BF16 = mybir.dt.bfloat16
NPBF16 = ml_dtypes.bfloat16

B, N, D, H = 8, 1024, 768, 12
HD = D // H            # 64
SCALE = float(np.sqrt(HD))
DT = D // 128          # 6  d-tiles
NT = N // 128          # 8  n(token)-tiles
NPAIR = H // 2         # 6 head pairs

XN_CHUNKS = [(0, 1), (1, 1), (2, 2), (4, 4)]   # (start nt, n nt)
C_PASSES = [(0, 1), (2, 3), (4, 5)]            # row groups per nt sweep
N_WARMUP = 22


def _chunks(lo, hi, step=512):
    out = []
    while lo < hi:
        out.append((lo, min(step, hi - lo)))
        lo += step
    return out


def _build_program():
    nc = bacc.Bacc(
        trn_type="TRN2", target_bir_lowering=False, debug=False, num_devices=B
    )
    xN_d = nc.dram_tensor("xN", [N, D], BF16, kind="ExternalInput").ap()
    xT_d = nc.dram_tensor("xT", [D, N], BF16, kind="ExternalInput").ap()
    wq_d = nc.dram_tensor("wq", [D, D], BF16, kind="ExternalInput").ap()
    wvT_d = nc.dram_tensor("wvT", [D, D], BF16, kind="ExternalInput").ap()
    wkT_d = nc.dram_tensor("wkT", [D, D], BF16, kind="ExternalInput").ap()
    wfcT_d = nc.dram_tensor("wfcT", [D, D], BF16, kind="ExternalInput").ap()
    bfc_d = nc.dram_tensor("bfc", [D], F32, kind="ExternalInput").ap()
    ident_d = nc.dram_tensor("ident", [128, 128], BF16, kind="ExternalInput").ap()
    outT_d = nc.dram_tensor("outT", [D, N], BF16, kind="ExternalOutput").ap()

    xN_r = xN_d.rearrange("(o p) d -> p o d", p=128)
    xT_r = xT_d.rearrange("(o p) n -> p o n", p=128)
    wq_r = wq_d.rearrange("(o p) d -> p o d", p=128)
    wvT_r = wvT_d.rearrange("(o p) j -> p o j", p=128)
    wkT_r = wkT_d.rearrange("(o p) j -> p o j", p=128)
    wfcT_r = wfcT_d.rearrange("(o p) e -> p o e", p=128)
    outT_r = outT_d.rearrange("(o p) n -> p o n", p=128)

    with tile.TileContext(nc) as tc:
        with tc.tile_pool(name="big", bufs=1) as big, \
             tc.tile_pool(name="outsp", bufs=3) as outsp, \
             tc.tile_pool(name="ps", bufs=6, space="PSUM") as ps, \
             tc.tile_pool(name="tp", bufs=2, space="PSUM") as tp:

            xN_sb = big.tile([128, NT, D], BF16, name="xN_sb")
            xT_sb = big.tile([128, DT, N], BF16, name="xT_sb")
            c_sb = big.tile([128, DT, D], BF16, name="c_sb")
            t1_sb = big.tile([128, DT, D], BF16, name="t1_sb")
            g2T_sb = big.tile([128, NPAIR, 128], BF16, name="g2T_sb")
            m_sb = big.tile([128, NPAIR, D], BF16, name="m_sb")
            weff_sb = big.tile([128, DT, D], BF16, name="weff_sb")
            wq_sb = big.tile([128, DT, D], BF16, name="wq_sb")
            wvT_sb = big.tile([128, DT, D], BF16, name="wvT_sb")
            wkT_sb = big.tile([128, DT, D], BF16, name="wkT_sb")
            wfcT_sb = big.tile([128, NPAIR, D], BF16, name="wfcT_sb")
            bias_sb = big.tile([128, DT], F32, name="bias_sb")
            ident_sb = big.tile([128, 128], BF16, name="ident_sb")

            # ---- input DMAs: xN chunks first, tiny ident, weights ----
            for nt0, nn in XN_CHUNKS:
                nc.sync.dma_start(xN_sb[:, nt0:nt0 + nn, :],
                                  xN_r[:, nt0:nt0 + nn, :])
            nc.sync.dma_start(ident_sb[:], ident_d)
            nc.sync.dma_start(wvT_sb[:], wvT_r)
            nc.sync.dma_start(wkT_sb[:], wkT_r)
            nc.sync.dma_start(wfcT_sb[:], wfcT_r)
            nc.sync.dma_start(wq_sb[:], wq_r)
            nc.sync.dma_start(bias_sb[:], bfc_d.rearrange("(o p) -> p o", p=128))
            nc.sync.dma_start(xT_sb[:], xT_r)

            nc.vector.memset(g2T_sb[:], 0.0)

            # ---- PE p-state warmup while waiting for the first xN chunk ---
            dummy_sb = big.tile([128, 128], BF16, name="dummy_sb")
            nc.vector.memset(dummy_sb[:], 0.0)
            for _ in range(N_WARMUP):
                wt = tp.tile([128, 128], BF16, tag="tp", name="wt")
                nc.tensor.transpose(wt[:], dummy_sb[:], dummy_sb[:])

            # ---- C = x^T x, upper blocks; nt-outer passes over row groups -
            # c_sb[p, i, c] = C[i*128+p, c]
            row_cols = {i: _chunks(i * 128, D) for i in range(DT)}
            row_psum = {}
            for rows in C_PASSES:
                for i in rows:
                    row_psum[i] = [
                        ps.tile([128, w], F32, tag="bank", name=f"pt_c{i}")
                        for (_, w) in row_cols[i]
                    ]
                for nt in range(NT):
                    for i in rows:
                        lhs = xN_sb[:, nt, i * 128:(i + 1) * 128]
                        for (off, w), pt in zip(row_cols[i], row_psum[i]):
                            nc.tensor.matmul(
                                pt[:], lhs, xN_sb[:, nt, off:off + w],
                                start=(nt == 0), stop=(nt == NT - 1),
                            )
                for i in rows:
                    for (off, w), pt in zip(row_cols[i], row_psum[i]):
                        nc.vector.tensor_copy(c_sb[:, i, off:off + w], pt[:])

            # lower block (j, i) = transpose of upper (i, j), i < j
            for i in range(DT):
                for j in range(i + 1, DT):
                    tpt = ps.tile([128, 128], BF16, tag="bank", name="tp_c")
                    nc.tensor.transpose(
                        tpt[:], c_sb[:, i, j * 128:(j + 1) * 128], ident_sb[:]
                    )
                    nc.vector.tensor_copy(
                        c_sb[:, j, i * 128:(i + 1) * 128], tpt[:]
                    )

            # ---- T1 = C @ wv^T : T1[d1, j'] = sum_d2 C[d2, d1] wvT[d2, j'] --
            for d1 in range(DT):
                cols = _chunks(0, D)
                pts = [ps.tile([128, w], F32, tag="bank", name="pt_t1")
                       for (_, w) in cols]
                for d2 in range(DT):
                    lhs = c_sb[:, d2, d1 * 128:(d1 + 1) * 128]
                    for (off, w), pt in zip(cols, pts):
                        nc.tensor.matmul(
                            pt[:], lhs, wvT_sb[:, d2, off:off + w],
                            start=(d2 == 0), stop=(d2 == DT - 1),
                        )
                for k, ((off, w), pt) in enumerate(zip(cols, pts)):
                    eng = nc.scalar if k == 0 else nc.vector
                    (eng.copy if k == 0 else eng.tensor_copy)(
                        t1_sb[:, d1, off:off + w], pt[:])

            # ---- GT per pair (gt = T1_pair^T @ wkT_pair), M one pair behind
            def emit_gt(t):
                gt = tp.tile([128, 128], F32, tag="tp", name="gt")
                pc = slice(t * 128, (t + 1) * 128)
                for dt in range(DT):
                    nc.tensor.matmul(
                        gt[:], t1_sb[:, dt, pc], wkT_sb[:, dt, pc],
                        start=(dt == 0), stop=(dt == DT - 1),
                    )
                nc.vector.tensor_copy(g2T_sb[0:64, t, 0:64], gt[0:64, 0:64])
                nc.vector.tensor_copy(g2T_sb[64:128, t, 64:128],
                                      gt[64:128, 64:128])

            def emit_m(t):
                cols = _chunks(0, D)
                for k, (off, w) in enumerate(cols):
                    pm = ps.tile([128, w], F32, tag="bank", name="pt_m")
                    nc.tensor.matmul(
                        pm[:], g2T_sb[:, t, :], wfcT_sb[:, t, off:off + w],
                        start=True, stop=True,
                    )
                    if k == 0:
                        nc.scalar.copy(m_sb[:, t, off:off + w], pm[:])
                    else:
                        nc.vector.tensor_copy(m_sb[:, t, off:off + w], pm[:])

            for t in range(NPAIR):
                emit_gt(t)
            for t in range(NPAIR):
                emit_m(t)

            # ---- Weff[d, e] = sum_j wq_s[j, d] M[j, e] ----
            for dt in range(DT):
                cols = _chunks(0, D)
                pts = [ps.tile([128, w], F32, tag="bank", name="pt_w")
                       for (_, w) in cols]
                for jt in range(DT):
                    lhs = wq_sb[:, jt, dt * 128:(dt + 1) * 128]
                    for (off, w), pt in zip(cols, pts):
                        nc.tensor.matmul(
                            pt[:], lhs, m_sb[:, jt, off:off + w],
                            start=(jt == 0), stop=(jt == DT - 1),
                        )
                for k, ((off, w), pt) in enumerate(zip(cols, pts)):
                    if k == 0:
                        nc.scalar.copy(weff_sb[:, dt, off:off + w], pt[:])
                    else:
                        nc.vector.tensor_copy(weff_sb[:, dt, off:off + w],
                                              pt[:])

            # ---- outT[e, n] = sum_d Weff[d, e] xT[d, n] + b[e] ----
            for et in range(DT):
                last = (et == DT - 1)
                cols = [(0, 512), (512, 384), (896, 128)] if last \
                    else [(0, 512), (512, 512)]
                pts = [ps.tile([128, w], F32, tag="bank", name="pt_o")
                       for (_, w) in cols]
                for dt in range(DT):
                    lhs = weff_sb[:, dt, et * 128:(et + 1) * 128]
                    for (off, w), pt in zip(cols, pts):
                        nc.tensor.matmul(
                            pt[:], lhs, xT_sb[:, dt, off:off + w],
                            start=(dt == 0), stop=(dt == DT - 1),
                        )
                ot = outsp.tile([128, N], BF16, tag="ot", name="ot")
                for k, ((off, w), pt) in enumerate(zip(cols, pts)):
                    if last and k == 1:
                        nc.vector.tensor_scalar_add(
                            ot[:, off:off + w], pt[:], bias_sb[:, et:et + 1])
                    else:
                        nc.scalar.add(ot[:, off:off + w], pt[:],
                                      bias_sb[:, et:et + 1])
                    if last:
                        eng = nc.scalar if k == 2 else nc.sync
                        eng.dma_start(outT_r[:, et, off:off + w],
                                      ot[:, off:off + w])
                if not last:
                    nc.sync.dma_start(outT_r[:, et, :], ot[:])

    nc.compile()
    return nc


_NC_CACHE = None
LAST_EXEC_NS = None
LAST_RES = None


def kernel(x, w_qkv, w_fc, b_fc, _trace=False):
    global _NC_CACHE, LAST_EXEC_NS, LAST_RES
    x = np.asarray(x, dtype=np.float32)
    w_qkv = np.asarray(w_qkv, dtype=np.float32)
    w_fc = np.asarray(w_fc, dtype=np.float32)
    b_fc = np.asarray(b_fc, dtype=np.float32)

    if _NC_CACHE is None:
        _NC_CACHE = _build_program()
    nc = _NC_CACHE

    wq = np.ascontiguousarray(SCALE * w_qkv[:D]).astype(NPBF16)
    wkT = np.ascontiguousarray(w_qkv[D:2 * D].T).astype(NPBF16)
    wvT = np.ascontiguousarray(w_qkv[2 * D:].T).astype(NPBF16)
    wfcT = np.ascontiguousarray(w_fc.T).astype(NPBF16)
    ident = np.eye(128, dtype=NPBF16)

    in_maps = []
    for b in range(B):
        in_maps.append({
            "xN": x[b].astype(NPBF16),
            "xT": np.ascontiguousarray(x[b].T).astype(NPBF16),
            "wq": wq, "wkT": wkT, "wvT": wvT, "wfcT": wfcT,
            "bfc": b_fc, "ident": ident,
        })

    res = bass_utils.run_bass_kernel_spmd(
        nc, in_maps, core_ids=list(range(B)), trace=_trace
    )
    LAST_EXEC_NS = res.exec_time_ns
    LAST_RES = res
    out = np.stack([res.results[b]["outT"].T.astype(np.float32)
                    for b in range(B)])
    return np.ascontiguousarray(out)
